# revision 1
# baseline (speedup 1.0000x reference)
"""GQA causal attention (B=2, T=2048, C=2048, 32 Q heads, 8 KV heads) on 8
Trainium2 NeuronCores — v4: row-sharded output projection with one
AllToAll per batch. Core i owns rows [i*256,(i+1)*256) of each batch's 2048
positions; the batch-0 exchange and out-projection overlap batch-1's
projection+attention, so only batch-1's ~27us out-proj plus one small
collective sit after the last attention chunk. Softmax normalization
broadcasts 1/l on the idle GpSimd engine instead of a PE matmul.

Sharding: tensor-parallel over KV-head groups for projections+attention
(core i owns KV head i and its 4 query heads), then ROW-sharded output
projection: core i owns t-slice [i*512, (i+1)*512) of B*T. The y exchange is
a single AllToAll of 2MB/core (vs 16MB AllGather in v1), and each core
computes all 2048 output channels for its own 512 rows.

Differences vs v1:
  - all matmul operands bf16 (fp32 PSUM accumulation) — rel err ~4e-3;
  - proj and attention fused per 512-t chunk (attention for chunk qc issues
    right after its projection) with per-chunk q/k/v tiles so Tile's dep
    tracking pipelines across phases;
  - causal diagonal trimmed: for diagonal k-blocks only the q >= k part of
    the 512-q range is computed (saves ~15% of attention matmul columns);
  - PSUM discipline: one 2-buf pool serves q0/q1/kv/vT passes sequentially,
    leaving banks for scores(2) + y(2) + norm(1) so nothing spills.
"""

import sys

sys.path.insert(0, "/opt/trn_rl_repo")

import numpy as np
import ml_dtypes

import concourse.bass as bass
import concourse.mybir as mybir
import concourse.tile as tile

P = 128
B, T, C = 2, 2048, 2048
BT = B * T            # 4096
NH, NKV = 32, 8
HD = C // NH          # 64
G = NH // NKV         # 4 q heads per kv head / per core
CQ = G * HD           # 256 q channels per core
KC = C // P           # 16 contraction chunks
TQ = 512              # t-chunk == per-core output row slice
NCORES = 8
NTB = BT // TQ        # 8 chunks; chunk tb has (b, qc) = (tb//4, tb%4)

f32 = mybir.dt.float32
f32r = mybir.dt.float32r
bf16 = mybir.dt.bfloat16
EXP = mybir.ActivationFunctionType.Exp
SCALE = float(HD) ** -0.5


def split_multi_waits(nc):
    """Walrus codegen allows only one sync-wait per engine instruction; move
    extras onto standalone same-engine EventSemaphore waits placed before."""
    for fn in nc.m.functions:
        for bb in fn.blocks:
            out = []
            for inst in bb.instructions:
                si = inst.sync_info
                if si is not None and si.on_wait and len(si.on_wait) > 1:
                    waits = list(si.on_wait)
                    for j, w in enumerate(waits[:-1]):
                        nop = mybir.InstEventSemaphore(
                            name=f"{inst.name}-ws{j}", ins=[], outs=[],
                            engine=inst.engine)
                        nop.sync_info = mybir.SyncInfo(on_wait=[w], on_update=[])
                        out.append(nop)
                    inst.sync_info = mybir.SyncInfo(
                        on_wait=[waits[-1]], on_update=list(si.on_update))
                out.append(inst)
            try:
                bb.instructions[:] = out
            except TypeError:
                bb.instructions.clear()
                bb.instructions.extend(out)


def build(reps=1, loop=0):
    nc = bass.Bass(num_devices=NCORES)

    xt_d = nc.dram_tensor("xt", [C, BT], bf16, kind="ExternalInput")
    wq_d = nc.dram_tensor("wq", [C, CQ], bf16, kind="ExternalInput")
    wkv_d = nc.dram_tensor("wkv", [C, P], bf16, kind="ExternalInput")
    wp_d = nc.dram_tensor("wp", [C, C], bf16, kind="ExternalInput")
    bpb_d = nc.dram_tensor("bpb", [P, C], f32, kind="ExternalInput")
    mask_d = nc.dram_tensor("masks", [P, P], bf16, kind="ExternalInput")
    ones_d = nc.dram_tensor("ones", [1, HD], f32r, kind="ExternalInput")
    idn_d = nc.dram_tensor("ident", [P, P], f32, kind="ExternalInput")
    vpad_d = nc.dram_tensor("vpad", [P, 2], bf16, kind="ExternalInput")
    out_d = nc.dram_tensor("out", [TQ, C], f32, kind="ExternalOutput")

    import contextlib
    with tile.TileContext(nc) as tc:
      for rep in range(reps):
       with (tc.For_i(0, loop, 1) if loop else contextlib.nullcontext()):
        with tc.tile_pool(name=f"res{rep}", bufs=1) as res, \
             tc.tile_pool(name=f"dram{rep}", bufs=1, space="DRAM") as dp:
            # weights / constants
            wq_sb = res.tile([P, KC, CQ], bf16, name=f"wq{rep}")
            nc.sync.dma_start(wq_sb[:], wq_d.rearrange("(o p) n -> p o n", p=P))
            wkv_sb = res.tile([P, KC, P], bf16, name=f"wkv{rep}")
            nc.sync.dma_start(wkv_sb[:], wkv_d.rearrange("(o p) n -> p o n", p=P))
            idn_sb = res.tile([P, P], f32, name=f"idn{rep}")
            mask_sb = res.tile([P, P], bf16, name=f"mk{rep}")
            ones_sb = res.tile([1, HD], f32r, name=f"on{rep}")
            bpb_sb = res.tile([P, C], f32, name=f"bp{rep}")
            wp_sb = res.tile([P, KC, C], bf16, name=f"wp{rep}")

            # per-chunk activations (separate tiles so attention on chunk qc
            # only depends on projections of chunks <= qc)
            qT = [[res.tile([HD, TQ], bf16, name=f"q{rep}_{h}_{tb}")
                   for tb in range(NTB)] for h in range(G)]
            kTt = [res.tile([HD, TQ], bf16, name=f"k{rep}_{tb}")
                   for tb in range(NTB)]
            va_t = [res.tile([P, TQ // P, HD + 2], bf16, name=f"v{rep}_{tb}")
                    for tb in range(NTB)]

            TH = TQ // 2   # 256-row t-window per core per batch
            ytl = [dp.tile([NCORES * CQ, TH], bf16, name=f"ytl{rep}_{bb}")
                   for bb in range(B)]
            yta = [dp.tile([NCORES * CQ, TH], bf16, name=f"yta{rep}_{bb}")
                   for bb in range(B)]

            with tc.tile_pool(name=f"xp{rep}", bufs=2) as xp, \
                 tc.tile_pool(name=f"pp{rep}", bufs=2, space="PSUM") as pp, \
                 tc.tile_pool(name=f"sps{rep}", bufs=2, space="PSUM") as sps, \
                 tc.tile_pool(name=f"yps{rep}", bufs=2, space="PSUM") as yps, \
                 tc.tile_pool(name=f"bps{rep}", bufs=1, space="PSUM") as bps, \
                 tc.tile_pool(name=f"ep{rep}", bufs=6) as ep, \
                 tc.tile_pool(name=f"np{rep}", bufs=2) as npo, \
                 tc.tile_pool(name=f"fp{rep}", bufs=2) as fp, \
                 tc.tile_pool(name=f"ops{rep}", bufs=1, space="PSUM") as ops:
                yts_tiles = {}

                def emit_exchange(bb):
                    nc.gpsimd.collective_compute(
                        "AllToAll", mybir.AluOpType.bypass,
                        replica_groups=[list(range(NCORES))],
                        ins=[ytl[bb][:].opt()], outs=[yta[bb][:].opt()])
                    yts = fp.tile([P, KC, TH], bf16, tag="yt")
                    for c in range(KC):
                        nc.sync.dma_start(yts[:, c, :],
                                          yta[bb][c * P:(c + 1) * P, :])
                    yts_tiles[bb] = yts

                def emit_outproj(bb):
                    yts = yts_tiles.pop(bb)
                    for tt in range(TH // P):
                        for occ in range(4):
                            o_ps = ops.tile([P, TQ], f32, tag="o",
                                            name=f"o{rep}_{bb}_{tt}_{occ}")
                            for c in range(KC):
                                nc.tensor.matmul(
                                    o_ps[:],
                                    yts[:, c, tt * P:(tt + 1) * P],
                                    wp_sb[:, c, occ * TQ:(occ + 1) * TQ],
                                    start=(c == 0), stop=(c == KC - 1))
                            o_sb = fp.tile([P, TQ], f32, tag="ob")
                            nc.vector.tensor_add(
                                o_sb[:], o_ps[:],
                                bpb_sb[:, occ * TQ:(occ + 1) * TQ])
                            nc.sync.dma_start(
                                out_d[bb * TH + tt * P:
                                      bb * TH + (tt + 1) * P,
                                      occ * TQ:(occ + 1) * TQ],
                                o_sb[:])
                for tb in range(NTB):
                    b, qc = tb // 4, tb % 4
                    # ---- projections for this 512-t chunk ----
                    xt_t = xp.tile([P, KC, TQ], bf16, tag="xt")
                    for c in range(KC):
                        nc.sync.dma_start(
                            xt_t[:, c, :],
                            xt_d[c * P:(c + 1) * P, tb * TQ:(tb + 1) * TQ])
                    if tb == 0:
                        # constants deferred behind the first x chunk so the
                        # first projection matmuls aren't DMA-queue-blocked
                        nc.sync.dma_start(idn_sb[:], idn_d[:, :])
                        nc.sync.dma_start(mask_sb[:], mask_d[:, :])
                        nc.sync.dma_start(ones_sb[:], ones_d[:, :])
                        nc.sync.dma_start(bpb_sb[:], bpb_d[:, :])
                    for k4 in range(TQ // P):
                        nc.sync.dma_start(va_t[tb][:, k4, HD:HD + 2],
                                          vpad_d[:, :])
                    # wp chunk loads spread over the first 4 chunks
                    # (first consumer is out-proj of batch 0 at tb==4)
                    if tb < 4:
                        for wchunk in range(4 * tb, 4 * tb + 4):
                            nc.sync.dma_start(
                                wp_sb[:, wchunk, :],
                                wp_d[wchunk * P:(wchunk + 1) * P, :])

                    q0_ps = pp.tile([P, TQ], f32, tag="pp")
                    for c in range(KC):
                        nc.tensor.matmul(q0_ps[:], wq_sb[:, c, 0:P],
                                         xt_t[:, c, :],
                                         start=(c == 0), stop=(c == KC - 1))
                    nc.vector.tensor_copy(qT[0][tb][:], q0_ps[0:HD, :])
                    nc.vector.tensor_copy(qT[1][tb][:], q0_ps[HD:P, :])
                    q1_ps = pp.tile([P, TQ], f32, tag="pp")
                    for c in range(KC):
                        nc.tensor.matmul(q1_ps[:], wq_sb[:, c, P:CQ],
                                         xt_t[:, c, :],
                                         start=(c == 0), stop=(c == KC - 1))
                    nc.vector.tensor_copy(qT[2][tb][:], q1_ps[0:HD, :])
                    nc.vector.tensor_copy(qT[3][tb][:], q1_ps[HD:P, :])
                    kv_ps = pp.tile([P, TQ], f32, tag="pp")
                    for c in range(KC):
                        nc.tensor.matmul(kv_ps[:], wkv_sb[:, c, :],
                                         xt_t[:, c, :],
                                         start=(c == 0), stop=(c == KC - 1))
                    nc.vector.tensor_copy(kTt[tb][:], kv_ps[0:HD, :])
                    vs_t = ep.tile([HD, TQ], f32, tag="vs", bufs=2)
                    nc.vector.tensor_copy(vs_t[:], kv_ps[HD:P, :])
                    for k4 in range(TQ // P):
                        vt_ps = pp.tile([P, HD], f32, tag="pp")
                        nc.tensor.transpose(vt_ps[:],
                                            vs_t[:, k4 * P:(k4 + 1) * P],
                                            idn_sb[0:HD, 0:HD])
                        nc.vector.tensor_copy(va_t[tb][:, k4, 0:HD], vt_ps[:])

                    # ---- attention for this chunk (all 4 heads) ----
                    nkb = 4 * qc + 4
                    for h in range(G):
                        y_ps = yps.tile([HD + 2, TQ], f32, tag="y")
                        for kb in range(nkb):
                            tb_k = b * 4 + kb // 4
                            j = kb - 4 * qc
                            qoff = max(0, j * P)
                            fr = TQ - qoff
                            s_ps = sps.tile([P, TQ], f32, tag="s")
                            nc.tensor.matmul(
                                s_ps[:, 0:fr],
                                kTt[tb_k][:, (kb % 4) * P:(kb % 4 + 1) * P],
                                qT[h][tb][:, qoff:TQ], start=True, stop=True)
                            ex = ep.tile([P, TQ], bf16, tag="ex")
                            if qoff:
                                nc.vector.memset(ex[:, 0:qoff], 0.0)
                            nc.scalar.activation(ex[:, qoff:TQ],
                                                 s_ps[:, 0:fr], EXP,
                                                 scale=SCALE)
                            if j >= 0:
                                nc.vector.tensor_mul(
                                    ex[:, qoff:qoff + P],
                                    ex[:, qoff:qoff + P], mask_sb[:])
                            nc.tensor.matmul(
                                y_ps[:], va_t[tb_k][:, kb % 4, :], ex[:],
                                start=(kb == 0), stop=(kb == nkb - 1))
                        rrow = npo.tile([1, TQ], f32r, tag="rr")
                        with nc.allow_low_precision(
                                reason="1/l in f32r (22-bit) is plenty"):
                            nc.vector.reciprocal(rrow[:], y_ps[HD:HD + 1, :])
                        bc_ps = bps.tile([HD, TQ], f32, tag="bc")
                        nc.tensor.matmul(bc_ps[:], ones_sb[:], rrow[:],
                                         start=True, stop=True)
                        ys = npo.tile([HD, TQ], bf16, tag="ys")
                        nc.vector.tensor_copy(ys[:], y_ps[0:HD, :])
                        yn = npo.tile([HD, TQ], bf16, tag="yn")
                        nc.vector.tensor_mul(yn[:], ys[:], bc_ps[:])
                        for half in range(2):
                            dst = (2 * qc + half) * CQ + h * HD
                            nc.sync.dma_start(
                                ytl[b][dst:dst + HD, :],
                                yn[:, half * TH:(half + 1) * TH])
                    if tb == 3:
                        # batch-0 y complete: exchange + stage now, out-proj
                        # after the next chunk so the collective hides
                        emit_exchange(0)

                # batch-1 exchange; batch-0's out-proj (ready since its
                # own exchange at tb==3) fills the PE while the collective
                # flies, then batch-1's out-proj closes the tail
                emit_exchange(1)
                emit_outproj(0)
                emit_outproj(1)

    split_multi_waits(nc)
    return nc


_NC_CACHE = None


def _get_nc():
    global _NC_CACHE
    if _NC_CACHE is None:
        _NC_CACHE = build()
    return _NC_CACHE


def make_in_maps(x, wq, wk, wv, wp, bp):
    x = np.asarray(x, dtype=np.float32)
    xt = np.ascontiguousarray(x.reshape(BT, C).T).astype(ml_dtypes.bfloat16)
    wp_b = np.ascontiguousarray(np.asarray(wp, np.float32)).astype(
        ml_dtypes.bfloat16)
    bpb = np.tile(np.asarray(bp, np.float32)[None, :], (P, 1))
    mask = np.triu(np.ones((P, P), np.float32)).astype(ml_dtypes.bfloat16)
    ident = np.eye(P, dtype=np.float32)
    vpad = np.zeros((P, 2), np.float32)
    vpad[:, 0] = 1.0
    vpad = vpad.astype(ml_dtypes.bfloat16)
    in_maps = []
    for i in range(NCORES):
        cs = slice(i * CQ, (i + 1) * CQ)
        ks = slice(i * HD, (i + 1) * HD)
        wkv = np.concatenate(
            [np.asarray(wk, np.float32)[:, ks],
             np.asarray(wv, np.float32)[:, ks]], axis=1)
        in_maps.append({
            "xt": xt,
            "wq": np.ascontiguousarray(
                np.asarray(wq, np.float32)[:, cs]).astype(ml_dtypes.bfloat16),
            "wkv": np.ascontiguousarray(wkv).astype(ml_dtypes.bfloat16),
            "wp": wp_b,
            "bpb": bpb,
            "masks": mask,
            "ones": np.ones((1, HD), np.float32),
            "ident": ident,
            "vpad": vpad,
        })
    return in_maps


def kernel(x, wq, wk, wv, wp, bp, _trace=False):
    from concourse.bass_utils import run_bass_kernel_spmd
    nc = _get_nc()
    in_maps = make_in_maps(x, wq, wk, wv, wp, bp)
    res = run_bass_kernel_spmd(nc, in_maps, list(range(NCORES)), trace=_trace)
    TH = TQ // 2
    out = np.empty((B, T, C), np.float32)
    for i in range(NCORES):
        o = res.results[i]["out"]
        for bb in range(B):
            out[bb, i * TH:(i + 1) * TH, :] = o[bb * TH:(bb + 1) * TH, :]
    if _trace:
        return out, res
    return out



# revision 7
# speedup vs baseline: 1.3294x; 1.3294x over previous
"""GQA causal attention (B=2, T=2048, C=2048, 32 Q heads, 8 KV heads) on 8
Trainium2 NeuronCores — v5.

Sharding: tensor-parallel over KV-head groups for projections+attention
(core i owns KV head i and its 4 query heads). Output projection is
row-sharded with INTERLEAVED ownership: for each exchange unit
u = (batch b, chunk-pair p) covering t-chunks qc=2p,2p+1, core i owns the
i-th 64-t slice of each chunk. One AllToAll per unit (4 total, 512 KiB
per core each) so only the last unit's exchange + one 128-row out-proj
sit after the final attention chunk.

v5 changes vs v4 (the big one is keeping the PE dense so the HAM clock
gate stays at 2.4 GHz instead of oscillating to 1.2):
  - attention inner loop interleaves TWO heads with an offset schedule
    [s(kb,h0); AV(kb-1,h1); s(kb,h1); AV(kb,h0)] so the PE never waits
    the ~550ns exp latency between a score and its AV matmul;
  - AV matmuls trimmed to the causal range [qoff:] on diagonal k-blocks
    (kills the ex memsets and ~10us of PE);
  - per-pair normalization: 1/l rows for both heads packed into one
    [2,512] tile, broadcast with a single selector matmul into one PSUM
    bank, multiplies deferred into the next pair's first iterations;
  - 4 collectives (one per unit) instead of 2 per-batch ones; staging
    DMAs ride the gpsimd SWDGE ring so they never head-block the SP ring
    (which caused a 22us PE stall in v4);
  - DMA consolidation: one descriptor per x chunk / yts stage, weights
    on the Act HWDGE ring, x+yn+out on the SP ring.
PSUM banks: pp(2, proj+outproj) + sps(2, scores) + yps(3, y accum) +
bps(1, 1/l broadcast) = 8.
"""

import sys

sys.path.insert(0, "/opt/trn_rl_repo")

import numpy as np
import ml_dtypes

import concourse.bass as bass
import concourse.mybir as mybir
import concourse.tile as tile

P = 128
B, T, C = 2, 2048, 2048
BT = B * T            # 4096
NH, NKV = 32, 8
HD = C // NH          # 64
G = NH // NKV         # 4 q heads per kv head / per core
CQ = G * HD           # 256 q channels per core
KC = C // P           # 16 contraction chunks
TQ = 512              # t-chunk
NCORES = 8
NTB = BT // TQ        # 8 chunks; chunk tb has (b, qc) = (tb//4, tb%4)
NU = 4                # exchange units: u = 2*b + p, chunk-pair p
UT = P                # t rows per core per unit (64 from each chunk)

f32 = mybir.dt.float32
f32r = mybir.dt.float32r
bf16 = mybir.dt.bfloat16
EXP = mybir.ActivationFunctionType.Exp
SCALE = float(HD) ** -0.5


def split_multi_waits(nc):
    """Walrus codegen allows only one sync-wait per engine instruction; move
    extras onto standalone same-engine EventSemaphore waits placed before."""
    for fn in nc.m.functions:
        for bb in fn.blocks:
            out = []
            for inst in bb.instructions:
                si = inst.sync_info
                if si is not None and si.on_wait and len(si.on_wait) > 1:
                    waits = list(si.on_wait)
                    for j, w in enumerate(waits[:-1]):
                        nop = mybir.InstEventSemaphore(
                            name=f"{inst.name}-ws{j}", ins=[], outs=[],
                            engine=inst.engine)
                        nop.sync_info = mybir.SyncInfo(on_wait=[w], on_update=[])
                        out.append(nop)
                    inst.sync_info = mybir.SyncInfo(
                        on_wait=[waits[-1]], on_update=list(si.on_update))
                out.append(inst)
            try:
                bb.instructions[:] = out
            except TypeError:
                bb.instructions.clear()
                bb.instructions.extend(out)


def build(reps=1, split=True):
    nc = bass.Bass(num_devices=NCORES)

    xt_d = nc.dram_tensor("xt", [C, BT], bf16, kind="ExternalInput")
    wq_d = nc.dram_tensor("wq", [C, CQ], bf16, kind="ExternalInput")
    wkv_d = nc.dram_tensor("wkv", [C, P], bf16, kind="ExternalInput")
    wp_d = nc.dram_tensor("wp", [C, C], bf16, kind="ExternalInput")
    bpb_d = nc.dram_tensor("bpb", [P, C], f32, kind="ExternalInput")
    mask_d = nc.dram_tensor("masks", [P, P], bf16, kind="ExternalInput")
    ones_d = nc.dram_tensor("ones", [1, HD], f32r, kind="ExternalInput")
    idn_d = nc.dram_tensor("ident", [P, P], f32, kind="ExternalInput")
    vpad_d = nc.dram_tensor("vpad", [P, (TQ // P) * 2], bf16,
                            kind="ExternalInput")
    out_d = nc.dram_tensor("out", [NU * UT, C], f32, kind="ExternalOutput")

    xt_v = xt_d.rearrange("(o p) n -> p o n", p=P)
    wq_v = wq_d.rearrange("(o p) n -> p o n", p=P)
    wkv_v = wkv_d.rearrange("(o p) n -> p o n", p=P)
    wp_v = wp_d.rearrange("(o p) n -> p o n", p=P)

    with tile.TileContext(nc) as tc:
      for rep in range(reps):
        with tc.tile_pool(name=f"res{rep}", bufs=1) as res, \
             tc.tile_pool(name=f"dram{rep}", bufs=1, space="DRAM") as dp:
            wq_sb = res.tile([P, KC, CQ], bf16, name=f"wq{rep}")
            wkv_sb = res.tile([P, KC, P], bf16, name=f"wkv{rep}")
            wp_sb = res.tile([P, KC, C], bf16, name=f"wp{rep}")
            bpb_sb = res.tile([P, C], f32, name=f"bp{rep}")
            idn_sb = res.tile([P, P], f32, name=f"idn{rep}")
            mask_sb = res.tile([P, P], bf16, name=f"mk{rep}")
            ones_sb = res.tile([1, HD], f32r, name=f"on{rep}")

            # weights & consts on the Act HWDGE ring (SP ring carries x)
            for hh in range(2):
                nc.scalar.dma_start(wq_sb[:, hh * 8:(hh + 1) * 8, :],
                                    wq_v[:, hh * 8:(hh + 1) * 8, :])
            nc.scalar.dma_start(wkv_sb[:], wkv_v[:, :, :])
            nc.scalar.dma_start(idn_sb[:], idn_d[:, :])
            nc.scalar.dma_start(mask_sb[:], mask_d[:, :])
            nc.scalar.dma_start(ones_sb[:], ones_d[:, :])

            qT = [[res.tile([HD, TQ], bf16, name=f"q{rep}_{h}_{tb}")
                   for tb in range(NTB)] for h in range(G)]
            kTt = [res.tile([HD, TQ], bf16, name=f"k{rep}_{tb}")
                   for tb in range(NTB)]
            va_t = [res.tile([P, TQ // P, HD + 2], bf16, name=f"v{rep}_{tb}")
                    for tb in range(NTB)]
            for tb in range(NTB):
                nc.scalar.dma_start(
                    va_t[tb][:, :, HD:HD + 2],
                    vpad_d.rearrange("p (k t) -> p k t", t=2))

            ytl = [dp.tile([NCORES * CQ, UT], bf16, name=f"ytl{rep}_{u}")
                   for u in range(NU)]
            yta = [dp.tile([NCORES * CQ, UT], bf16, name=f"yta{rep}_{u}")
                   for u in range(NU)]

            with tc.tile_pool(name=f"xp{rep}", bufs=2) as xp, \
                 tc.tile_pool(name=f"pp{rep}", bufs=2, space="PSUM") as pp, \
                 tc.tile_pool(name=f"sps{rep}", bufs=2, space="PSUM") as sps, \
                 tc.tile_pool(name=f"yps{rep}", bufs=3, space="PSUM") as yps, \
                 tc.tile_pool(name=f"bps{rep}", bufs=1, space="PSUM") as bps, \
                 tc.tile_pool(name=f"ep{rep}", bufs=6) as ep, \
                 tc.tile_pool(name=f"np{rep}", bufs=3) as npo, \
                 tc.tile_pool(name=f"fp{rep}", bufs=2) as fp, \
                 tc.tile_pool(name=f"vp{rep}", bufs=2) as vp:
                yts_tiles = {}
                pending = []   # deferred norm closures: (kind, fn)

                def flush_pending():
                    for _, fn in pending:
                        fn()
                    pending.clear()

                def emit_norm(b, qc, hp, y0, y1):
                    """recip + PSUM->SBUF copies now (frees the y banks);
                    bc matmuls + normalize muls + ytl dmas deferred into the
                    next pair's first iterations (PE slots)."""
                    h0, h1 = 2 * hp, 2 * hp + 1
                    u, c2 = 2 * b + qc // 2, qc % 2
                    rys = []
                    for hi, y_ps in ((0, y0), (1, y1)):
                        rr = npo.tile([1, TQ], f32r, tag=f"rr{hi}",
                                      name=f"rr{rep}")
                        with nc.allow_low_precision(
                                reason="1/l in f32r (22-bit) is plenty"):
                            nc.vector.reciprocal(rr[:], y_ps[HD:HD + 1, :])
                        ys = npo.tile([HD, TQ], bf16, tag=f"ys{hi}",
                                      name=f"ys{rep}")
                        nc.vector.tensor_copy(ys[:], y_ps[0:HD, :])
                        rys.append((rr, ys))

                    def do_head(hi):
                        rr, ys = rys[hi]
                        h = h0 if hi == 0 else h1
                        bc = bps.tile([HD, TQ], f32, tag="bc",
                                      name=f"bc{rep}")
                        nc.tensor.matmul(bc[:], ones_sb[:], rr[:],
                                         start=True, stop=True)
                        yn = npo.tile([HD, TQ], bf16, tag=f"yn{hi}",
                                      name=f"yn{rep}")
                        nc.vector.tensor_mul(yn[:], ys[:], bc[:])
                        dst = ytl[u].rearrange(
                            "(j ch) (c2 t) -> ch j c2 t", j=NCORES, c2=2)
                        nc.sync.dma_start(
                            dst[h * HD:(h + 1) * HD, :, c2, :],
                            yn[:].rearrange("d (j t) -> d j t", j=NCORES))

                    pending.append(("h0", lambda: do_head(0)))
                    pending.append(("h1", lambda: do_head(1)))

                def emit_exchange(u):
                    nc.gpsimd.collective_compute(
                        "AllToAll", mybir.AluOpType.bypass,
                        replica_groups=[list(range(NCORES))],
                        ins=[ytl[u][:].opt()], outs=[yta[u][:].opt()])
                    yts = fp.tile([P, KC, UT], bf16, tag="yt", name=f"yt{rep}")
                    nc.gpsimd.dma_start(
                        yts[:], yta[u].rearrange("(c p) t -> p c t", p=P))
                    yts_tiles[u] = yts

                def emit_outproj(u):
                    yts = yts_tiles.pop(u)
                    for occ in range(4):
                        o_ps = pp.tile([P, TQ], f32, tag="pp",
                                       name=f"o{rep}_{u}_{occ}")
                        for c in range(KC):
                            nc.tensor.matmul(
                                o_ps[:], yts[:, c, :],
                                wp_sb[:, c, occ * TQ:(occ + 1) * TQ],
                                start=(c == 0), stop=(c == KC - 1))
                        o_sb = fp.tile([P, TQ], f32, tag="ob", name=f"ob{rep}")
                        nc.vector.tensor_add(
                            o_sb[:], o_ps[:],
                            bpb_sb[:, occ * TQ:(occ + 1) * TQ])
                        nc.sync.dma_start(
                            out_d[u * P:(u + 1) * P,
                                  occ * TQ:(occ + 1) * TQ],
                            o_sb[:])

                for tb in range(NTB):
                    b, qc = tb // 4, tb % 4
                    # ---- x chunk load (SP ring, one descriptor) ----
                    xt_t = xp.tile([P, KC, TQ], bf16, tag="xt")
                    if tb == 0:
                        for i in range(4):
                            nc.sync.dma_start(
                                xt_t[:, 4 * i:4 * i + 4, :],
                                xt_v[:, 4 * i:4 * i + 4,
                                     tb * TQ:(tb + 1) * TQ])
                    else:
                        nc.sync.dma_start(
                            xt_t[:], xt_v[:, :, tb * TQ:(tb + 1) * TQ])
                    # ---- projections ----
                    q0_ps = pp.tile([P, TQ], f32, tag="pp")
                    for c in range(KC):
                        nc.tensor.matmul(q0_ps[:], wq_sb[:, c, 0:P],
                                         xt_t[:, c, :],
                                         start=(c == 0), stop=(c == KC - 1))
                    nc.vector.tensor_copy(qT[0][tb][:], q0_ps[0:HD, :])
                    nc.vector.tensor_copy(qT[1][tb][:], q0_ps[HD:P, :])
                    q1_ps = pp.tile([P, TQ], f32, tag="pp")
                    for c in range(KC):
                        nc.tensor.matmul(q1_ps[:], wq_sb[:, c, P:CQ],
                                         xt_t[:, c, :],
                                         start=(c == 0), stop=(c == KC - 1))
                    nc.vector.tensor_copy(qT[2][tb][:], q1_ps[0:HD, :])
                    nc.vector.tensor_copy(qT[3][tb][:], q1_ps[HD:P, :])
                    kv_ps = pp.tile([P, TQ], f32, tag="pp")
                    for c in range(KC):
                        nc.tensor.matmul(kv_ps[:], wkv_sb[:, c, :],
                                         xt_t[:, c, :],
                                         start=(c == 0), stop=(c == KC - 1))
                    nc.vector.tensor_copy(kTt[tb][:], kv_ps[0:HD, :])
                    vs_t = vp.tile([HD, TQ], f32, tag="vs")
                    nc.vector.tensor_copy(vs_t[:], kv_ps[HD:P, :])
                    for k4 in range(TQ // P):
                        vt_ps = pp.tile([P, HD], f32, tag="pp")
                        nc.tensor.transpose(vt_ps[:],
                                            vs_t[:, k4 * P:(k4 + 1) * P],
                                            idn_sb[0:HD, 0:HD])
                        nc.vector.tensor_copy(va_t[tb][:, k4, 0:HD], vt_ps[:])

                    # ---- attention: head pairs with offset schedule ----
                    nkb = 4 * qc + 4
                    for hp in range(2):
                        h0, h1 = 2 * hp, 2 * hp + 1
                        y0 = yps.tile([HD + 2, TQ], f32, tag="y",
                                      name=f"y{rep}_{tb}_{h0}")
                        y1 = yps.tile([HD + 2, TQ], f32, tag="y",
                                      name=f"y{rep}_{tb}_{h1}")
                        exs = {}

                        def emit_score(kb, h):
                            j = kb - 4 * qc
                            qoff = max(0, j * P)
                            fr = TQ - qoff
                            tb_k = b * 4 + kb // 4
                            s_ps = sps.tile([P, TQ], f32, tag="s", name=f"s{rep}")
                            nc.tensor.matmul(
                                s_ps[:, 0:fr],
                                kTt[tb_k][:, (kb % 4) * P:(kb % 4 + 1) * P],
                                qT[h][tb][:, qoff:TQ], start=True, stop=True)
                            ex = ep.tile([P, TQ], bf16, tag="ex", name=f"ex{rep}")
                            nc.scalar.activation(ex[:, qoff:TQ],
                                                 s_ps[:, 0:fr], EXP,
                                                 scale=SCALE)
                            if j >= 0:
                                nc.vector.tensor_mul(
                                    ex[:, qoff:qoff + P],
                                    ex[:, qoff:qoff + P], mask_sb[:])
                            exs[(kb, h)] = (ex, qoff)

                        def emit_av(kb, h, y_ps):
                            ex, qoff = exs.pop((kb, h))
                            tb_k = b * 4 + kb // 4
                            nc.tensor.matmul(
                                y_ps[:, qoff:TQ],
                                va_t[tb_k][:, kb % 4, :], ex[:, qoff:TQ],
                                start=(kb == 0), stop=(kb == nkb - 1))

                        for kb in range(nkb):
                            emit_score(kb, h0)
                            if kb == 0 and pending:
                                pending[0][1]()   # bc matmul of prev pair
                                del pending[0]
                            if kb > 0:
                                if kb == 1 and pending:
                                    flush_pending()   # yn muls + dmas
                                emit_av(kb - 1, h1, y1)
                            emit_score(kb, h1)
                            emit_av(kb, h0, y0)
                        emit_av(nkb - 1, h1, y1)
                        emit_norm(b, qc, hp, y0, y1)

                    # wp spread over chunks 0-3 on the Act ring, emitted
                    # after the chunk's exps so they don't delay attention;
                    # first consumer is out-proj of unit 0 at tb==3
                    if tb < 4:
                        nc.scalar.dma_start(
                            wp_sb[:, 4 * tb:4 * tb + 4, :],
                            wp_v[:, 4 * tb:4 * tb + 4, :])
                    if tb == 1:
                        nc.scalar.dma_start(bpb_sb[:], bpb_d[:, :])

                    # ---- unit boundaries ----
                    if tb % 2 == 1:
                        flush_pending()
                        emit_exchange(2 * b + qc // 2)
                    if tb == 3:
                        emit_outproj(0)
                    elif tb == 5:
                        emit_outproj(1)
                    elif tb == 6:
                        emit_outproj(2)
                    elif tb == 7:
                        emit_outproj(3)

    if split:
        split_multi_waits(nc)
    return nc


_NC_CACHE = None


def _get_nc():
    global _NC_CACHE
    if _NC_CACHE is None:
        _NC_CACHE = build()
    return _NC_CACHE


def make_in_maps(x, wq, wk, wv, wp, bp):
    x = np.asarray(x, dtype=np.float32)
    xt = np.ascontiguousarray(x.reshape(BT, C).T).astype(ml_dtypes.bfloat16)
    wp_b = np.ascontiguousarray(np.asarray(wp, np.float32)).astype(
        ml_dtypes.bfloat16)
    bpb = np.tile(np.asarray(bp, np.float32)[None, :], (P, 1))
    mask = np.triu(np.ones((P, P), np.float32)).astype(ml_dtypes.bfloat16)
    ident = np.eye(P, dtype=np.float32)
    vpad = np.zeros((P, TQ // P, 2), np.float32)
    vpad[:, :, 0] = 1.0
    vpad = vpad.reshape(P, -1).astype(ml_dtypes.bfloat16)
    in_maps = []
    for i in range(NCORES):
        cs = slice(i * CQ, (i + 1) * CQ)
        ks = slice(i * HD, (i + 1) * HD)
        wkv = np.concatenate(
            [np.asarray(wk, np.float32)[:, ks],
             np.asarray(wv, np.float32)[:, ks]], axis=1)
        in_maps.append({
            "xt": xt,
            "wq": np.ascontiguousarray(
                np.asarray(wq, np.float32)[:, cs]).astype(ml_dtypes.bfloat16),
            "wkv": np.ascontiguousarray(wkv).astype(ml_dtypes.bfloat16),
            "wp": wp_b,
            "bpb": bpb,
            "masks": mask,
            "ones": np.ones((1, HD), np.float32),
            "ident": ident,
            "vpad": vpad,
        })
    return in_maps


def kernel(x, wq, wk, wv, wp, bp, _trace=False):
    from concourse.bass_utils import run_bass_kernel_spmd
    nc = _get_nc()
    in_maps = make_in_maps(x, wq, wk, wv, wp, bp)
    res = run_bass_kernel_spmd(nc, in_maps, list(range(NCORES)), trace=_trace)
    out = np.empty((B, T, C), np.float32)
    for i in range(NCORES):
        o = res.results[i]["out"]       # [NU*UT, C]
        for u in range(NU):
            bb, p = u // 2, u % 2
            for c2 in range(2):
                t0 = (2 * p + c2) * TQ + i * HD
                out[bb, t0:t0 + HD, :] = \
                    o[u * P + c2 * HD:u * P + (c2 + 1) * HD, :]
    if _trace:
        return out, res
    return out
